# revision 38
# baseline (speedup 1.0000x reference)
"""Multi-head attention (B=2,S=4096,E=768,H=12,D=64 + 16-token K/V prompt
prefix) on 8 Trainium2 NeuronCores.

Sharding: 2 batches x 4 head-groups (3 heads each). Each core computes QKV
projections for its 3 heads, full attention over its batch, and a partial
output projection (its 192 ctx channels); the host sums the 4 partials per
batch.

v2 design (vs the 485us baseline, which was jointly PE- and ScalarE-bound):
  * scores matmuls run in fp8e4m3 with MatmulPerfMode.DoubleRow (d=64 split
    as [32 partitions x 2 interleave]); 0.5 cycles/row halves scores PE time.
    q/k live only in fp8; measured end-to-end rel-err impact ~1.2e-2.
  * ctx matmul is flipped: expt [k,128q] tiles are the *stationary* operand
    and v [k,65] the moving one, so each instruction streams 65 rows instead
    of 512 -- ctx PE time halves.  The ones-column in v still accumulates
    the softmax denominator (psc column 64).
  * exp is the 1/8-scaled softmax numerator; it is load-balanced across
    ScalarE (activation Exp, scale=1/8) AND Vector/Pool engines
    (tensor_tensor pow: expt = (e^{1/8})^s with a memset base tile).
  * ctx comes out of PSUM in [q, d] orientation; normalization is a single
    per-partition tensor_scalar divide; re-transposition to [d, q] for the
    out-projection rides the idle DMA engines via XBAR dma_start_transpose
    (two heads batched per transfer to satisfy the 128-col constraint).
  * PSUM: 2x[128,1024] scores + 1x[128,2,4,128pad] ctx + 2x[128,512]
    time-multiplexed (v-proj/bg q-proj/out-proj) = 8 banks exactly.
"""

import sys
import threading

import numpy as np

if "/opt/trn_rl_repo" not in sys.path:
    sys.path.insert(0, "/opt/trn_rl_repo")

import ml_dtypes

BF16 = ml_dtypes.bfloat16
FP8 = ml_dtypes.float8_e4m3

B, S, E, H, D, PP = 2, 4096, 768, 12, 64, 16
NCORES = 8
NG = 4          # head-groups (tensor parallel)
HL = H // NG    # 3 local heads
CL = HL * D     # 192 local channels
SKV = PP + S    # 4112
NKT = S // 128  # 32 full k-tiles (prefix handled separately)
QT = 1024       # q tile width for scores/exp/ctx
NSQ = S // QT   # 4
TRAIL = 12      # ctx matmuls trail scores by this many slots
NST = S // 128  # 32 v stiles
GAP = 6
# Schraudolph exp for the DVE share: bf16 bits of exp(s/8) ~=
# int16(s*SCHR_A + SCHR_B); one fused tensor_scalar (mult,add) writing
# through an int16 bitcast of the bf16 expt tile.  ~1.8% rms relative
# error on those tiles; the Act share stays exact, so total error scales
# with sqrt(phi).  C=7.5 centers the sawtooth; +0.5 makes trunc rounding.
SCHR_A = 128 * 1.4426950408889634 / 8   # 128*log2(e)/8
SCHR_B = 16256.5 - 7.5
# exp engine assignment pattern per slot: A=ScalarE (exact), D=Vector
# (Schraudolph).  GPSIMD cannot access PSUM; DVE has no transcendentals.
EXP_PAT = "ADADADADA"

_lock = threading.Lock()
_compiled = {}


def _build():
    import concourse.bass as bass  # noqa: F401
    import concourse.mybir as mybir
    import concourse.tile as tile
    from concourse import bacc

    f32 = mybir.dt.float32
    bf16 = mybir.dt.bfloat16
    fp8 = mybir.dt.float8e4
    i16 = mybir.dt.int16
    EXP = mybir.ActivationFunctionType.Exp
    IDN = mybir.ActivationFunctionType.Identity
    DIV = mybir.AluOpType.divide
    MUL = mybir.AluOpType.mult
    ADD = mybir.AluOpType.add
    DR = mybir.MatmulPerfMode.DoubleRow

    nc = bacc.Bacc("TRN2", target_bir_lowering=False, debug=False)

    xqT = nc.dram_tensor("xqT", [E, S], bf16, kind="ExternalInput").ap()
    xkT = nc.dram_tensor("xkT", [E, S], bf16, kind="ExternalInput").ap()
    xvT = nc.dram_tensor("xvT", [E, S], bf16, kind="ExternalInput").ap()
    wqT = nc.dram_tensor("wqT", [E, CL], bf16, kind="ExternalInput").ap()
    wkT = nc.dram_tensor("wkT", [E, CL], bf16, kind="ExternalInput").ap()
    wvT = nc.dram_tensor("wvT", [E, CL], bf16, kind="ExternalInput").ap()
    woT = nc.dram_tensor("woT", [CL, E], bf16, kind="ExternalInput").ap()
    bq = nc.dram_tensor("bq", [96, 2], f32, kind="ExternalInput").ap()
    bk = nc.dram_tensor("bk", [96, 2], f32, kind="ExternalInput").ap()
    bv = nc.dram_tensor("bv", [1, CL], f32, kind="ExternalInput").ap()
    kp8 = nc.dram_tensor("kp8", [96, 2, PP], fp8, kind="ExternalInput").ap()
    vp = nc.dram_tensor("vp", [PP, HL, D + 1], bf16, kind="ExternalInput").ap()
    outT = nc.dram_tensor("outT", [E, S], f32, kind="ExternalOutput").ap()

    with tile.TileContext(nc) as tc:
        with tc.tile_pool(name="persist", bufs=1) as pers:
            # q-projection weights/bias first: they gate the first matmuls
            wq_sb = pers.tile([128, 6, CL], bf16)
            nc.sync.dma_start(wq_sb[:], wqT.rearrange("(t p) c -> p t c", p=128))
            bq_sb = pers.tile([96, 2], f32)
            nc.sync.dma_start(bq_sb[:], bq[:])

            wk_sb = pers.tile([128, 6, CL], bf16)
            wv_sb = pers.tile([128, 6, CL], bf16)
            wo_sb = pers.tile([128, 2, E], bf16)
            bk_sb = pers.tile([96, 2], f32)
            bvb_sb = pers.tile([128, CL], f32)
            kp_sb = pers.tile([96, 2, PP], fp8)
            vp_sb = pers.tile([PP, HL, D + 1], bf16)

            # activations
            qT8 = pers.tile([96, 2, S], fp8)
            kT8 = pers.tile([96, 2, S], fp8)
            v_sb = pers.tile([128, NST, HL, D + 1], bf16)
            ctxT_sb = pers.tile([128, 2, S], bf16)
            expp_sb = pers.tile([PP, HL, S], bf16)  # prefix exp rows per head
            # normalized ctx staging, [q, d] orientation, manual sq-parity
            # double buffer; cn01 packs heads 0,1 so one XBAR dma transposes
            # 128 columns at once; cn2 pads head 2 with a junk half.
            cn01 = pers.tile([128, 2, 8, 2, D], bf16)
            cn2 = pers.tile([128, 2, 8, 2, D], bf16)

            nc.vector.memset(v_sb[:, :, :, D:D + 1], 1.0)
            nc.vector.memset(cn2[:], 0.0)

            # One unified stream phase.  PSUM: ps_s 3x[128,1024] (6 banks,
            # 3-deep rotation so the exp engines run back-to-back) + ps_c
            # 1x[128,2,4,128] (2 banks) = 8.  Projections / out-projection /
            # prefix borrow ps_s rotation slots (same tag+shape, partial use);
            # note matmul PSUM writes must stay within one 2KB bank, so every
            # matmul writes at most 512 f32 columns.
            with (
                tc.tile_pool(name="ps_s", bufs=3, space="PSUM") as ps_s,
                tc.tile_pool(name="ps_c", bufs=1, space="PSUM") as ps_c,
                tc.tile_pool(name="expt_pool", bufs=20) as expt_pool,
                tc.tile_pool(name="xv_pool", bufs=8) as xv_pool,
                tc.tile_pool(name="xq2_pool", bufs=26) as xq2_pool,
                tc.tile_pool(name="out_pool", bufs=3) as out_pool,
                tc.tile_pool(name="nrm_pool", bufs=2) as nrm_pool,
            ):
                expcnt = [0]

                def emit_exp(dst, src, exact=False):
                    eng = "A" if exact else EXP_PAT[expcnt[0] % len(EXP_PAT)]
                    expcnt[0] += 1
                    if eng == "A":
                        nc.scalar.activation(dst, src, EXP, scale=0.125)
                    else:
                        nc.vector.tensor_scalar(
                            dst.bitcast(i16), src, float(SCHR_A),
                            float(SCHR_B), MUL, ADD)

                def pss_tile(name):
                    return ps_s.tile([128, QT], f32, tag="pss", name=name)

                def load_x_chunks(xin, sq):
                    tiles = []
                    for ech in range(6):
                        xt = xq2_pool.tile([128, QT], bf16, tag="xt2",
                                           name="xt2")
                        nc.sync.dma_start(
                            xt[:],
                            xin[ech * 128:(ech + 1) * 128,
                                sq * QT:(sq + 1) * QT],
                        )
                        tiles.append(xt)
                    return tiles

                def emit_proj_group(xts, wsb, bsb, dst8, sq, i):
                    # one [96, 1024] projection result via two 512-col mms
                    p = pss_tile("pqk")
                    for n in range(2):
                        ns = slice(n * 512, (n + 1) * 512)
                        for ech in range(6):
                            nc.tensor.matmul(
                                p[0:96, ns],
                                wsb[:, ech, i * 96:(i + 1) * 96],
                                xts[ech][:, ns],
                                start=(ech == 0), stop=(ech == 5),
                            )
                    qs = slice(sq * QT, (sq + 1) * QT)
                    nc.scalar.activation(
                        dst8[0:96, i, qs], p[0:96, :], IDN,
                        bias=bsb[:, i:i + 1])

                def emit_prefix(sq, h):
                    hp = slice(32 * h, 32 * h + 32)
                    psp = pss_tile("psp")
                    for n in range(2):
                        ns = slice(n * 512, (n + 1) * 512)
                        qs = slice(sq * QT + n * 512, sq * QT + (n + 1) * 512)
                        nc.tensor.matmul(
                            psp[0:PP, ns], kp_sb[hp, :, :], qT8[hp, :, qs],
                            start=True, stop=True, perf_mode=DR,
                        )
                    emit_exp(expp_sb[:, h, sq * QT:(sq + 1) * QT],
                             psp[0:PP, :], exact=True)

                # xv DMA loads, one sq-group of 6 chunks at a time
                xvts = {}

                def load_xv(sqx):
                    tiles = []
                    for ech in range(6):
                        xvt = xv_pool.tile([128, QT], bf16, tag="xvt",
                                           name="xvt")
                        nc.sync.dma_start(
                            xvt[:],
                            xvT[ech * 128:(ech + 1) * 128,
                                sqx * QT:(sqx + 1) * QT],
                        )
                        tiles.append(xvt)
                    xvts[sqx] = tiles

                def emit_vproj(st):
                    sqx, stl = st // (QT // 128), st % (QT // 128)
                    if st == 0:
                        load_xv(0)
                    # prefetch mid-group so the xv DMAs stay off the k-proj
                    # chunks' critical DMA window at stream start
                    if stl == 4 and sqx + 1 < NSQ:
                        load_xv(sqx + 1)
                    pv = pss_tile("pv")
                    for ech in range(6):
                        nc.tensor.matmul(
                            pv[:, 0:CL],
                            xvts[sqx][ech][:, stl * 128:(stl + 1) * 128],
                            wv_sb[:, ech, :],
                            start=(ech == 0), stop=(ech == 5),
                        )
                    nc.vector.tensor_add(
                        v_sb[:, st, :, 0:D],
                        pv[:, 0:CL].rearrange("p (h d) -> p h d", h=HL),
                        bvb_sb[:].rearrange("p (h d) -> p h d", h=HL),
                    )
                    if stl == (QT // 128) - 1:
                        del xvts[sqx]

                def emit_scores_exp(sq, h, kt):
                    hp = slice(32 * h, 32 * h + 32)
                    expt = expt_pool.tile([128, QT], bf16, tag="expt",
                                          name="expt")
                    pss = pss_tile("pss")
                    for n in range(2):
                        ns = slice(n * 512, (n + 1) * 512)
                        qs = slice(sq * QT + n * 512, sq * QT + (n + 1) * 512)
                        nc.tensor.matmul(
                            pss[:, ns],
                            kT8[hp, :, kt * 128:(kt + 1) * 128],
                            qT8[hp, :, qs],
                            start=True, stop=True, perf_mode=DR,
                        )
                    emit_exp(expt[:], pss[:])
                    return expt

                psc_tiles = {}
                outproj_work = []

                def emit_ctx(sq, h, kt, expt):
                    key = (sq, h)
                    if kt == 0:
                        psc_tiles[key] = ps_c.tile(
                            [128, 2, 4, 128], f32, tag="psc", name="psc")
                    psc = psc_tiles[key]
                    # PSUM zero-region (2KB bank) semantics: only the first
                    # slice per bank may carry start=True (it marks the whole
                    # region pending-zero; sibling slices' first writes then
                    # overwrite-on-first-touch), and only the last slice may
                    # carry stop=True (it clears the whole region's group).
                    for qb in range(8):
                        nc.tensor.matmul(
                            psc[:, qb // 4, qb % 4, 0:D + 1],
                            expt[:, qb * 128:(qb + 1) * 128],
                            v_sb[:, kt, h, :],
                            start=(kt == 0 and qb % 4 == 0),
                            stop=(kt == NKT - 1 and qb % 4 == 3),
                        )
                    if kt == TRAIL - 1:
                        # prompt-prefix ctx contribution (reads expp_sb rows)
                        for qb in range(8):
                            qs = slice(sq * QT + qb * 128,
                                       sq * QT + (qb + 1) * 128)
                            nc.tensor.matmul(
                                psc[:, qb // 4, qb % 4, 0:D + 1],
                                expp_sb[:, h, qs],
                                vp_sb[:, h, :],
                                start=False, stop=False,
                            )
                    if kt == NKT - 1:
                        emit_norm(sq, h, psc)
                        del psc_tiles[key]

                def emit_norm(sq, h, psc):
                    par = sq % 2
                    cn = cn2 if h == 2 else cn01
                    hh = 0 if h == 2 else h
                    # hw tensor_scalar has no divide: batched reciprocal of
                    # the 8 denominator columns, then per-block multiplies,
                    # alternating Act/DVE so neither engine queue bursts
                    rc = nrm_pool.tile([128, 8], f32, tag="rc", name="rc")
                    nc.vector.reciprocal(
                        rc[:].rearrange("p (a b) -> p a b", a=2),
                        psc[:, :, :, D:D + 1].squeeze(3))
                    for qb in range(8):
                        if qb % 2 == 0:
                            nc.scalar.activation(
                                cn[:, par, qb, hh, :],
                                psc[:, qb // 4, qb % 4, 0:D],
                                IDN, scale=rc[:, qb:qb + 1])
                        else:
                            nc.vector.tensor_scalar(
                                cn[:, par, qb, hh, :],
                                psc[:, qb // 4, qb % 4, 0:D],
                                rc[:, qb:qb + 1],
                                None, MUL,
                            )
                    if h >= 1:
                        # heads 0,1 pair (after h1) / head 2 -> XBAR transpose
                        cnin, pr = (cn01, 0) if h == 1 else (cn2, 1)
                        for qb in range(8):
                            qs = slice(sq * QT + qb * 128,
                                       sq * QT + (qb + 1) * 128)
                            nc.sync.dma_start_transpose(
                                ctxT_sb[:, pr, qs], cnin[:, par, qb, :, :])
                    if h == HL - 1:
                        for et in range(6):
                            for n in range(2):
                                outproj_work.append((et, sq, n))

                def emit_outproj_tile(et, sq, n):
                    es = slice(et * 128, (et + 1) * 128)
                    qs = slice(sq * QT + n * 512, sq * QT + (n + 1) * 512)
                    po3 = pss_tile("po3")
                    nc.tensor.matmul(
                        po3[:, 0:512], wo_sb[:, 0, es], ctxT_sb[:, 0, qs],
                        start=True, stop=False,
                    )
                    nc.tensor.matmul(
                        po3[:, 0:512], wo_sb[0:64, 1, es],
                        ctxT_sb[0:64, 1, qs],
                        start=False, stop=True,
                    )
                    # DMA cannot read PSUM: stage through SBUF via DVE
                    ot = out_pool.tile([128, 512], f32, tag="ot", name="ot")
                    nc.vector.tensor_copy(ot[:], po3[:, 0:512])
                    nc.sync.dma_start(outT[es, qs], ot[:])

                # ---- stream startup: q-proj(sq0), prefix(sq0), k-proj(0) ----
                # DMA issue order matters: the (serial) DMA engines gate the
                # startup ramp.  xq0/xk0 first (they gate the first exps),
                # then wv/vp/xv-side, then the remaining xk blocks; wo last.
                xq0 = load_x_chunks(xqT, 0)
                nc.sync.dma_start(
                    wk_sb[:], wkT.rearrange("(t p) c -> p t c", p=128))
                nc.sync.dma_start(bk_sb[:], bk[:])
                nc.sync.dma_start(kp_sb[:], kp8[:])
                for i in range(2):
                    emit_proj_group(xq0, wq_sb, bq_sb, qT8, 0, i)
                xk = {0: load_x_chunks(xkT, 0)}
                nc.sync.dma_start(
                    wv_sb[:], wvT.rearrange("(t p) c -> p t c", p=128))
                nc.sync.dma_start(bvb_sb[:], bv.to_broadcast((128, CL)))
                nc.sync.dma_start(vp_sb[:], vp[:])
                for h in range(HL):
                    emit_prefix(0, h)
                for i in range(2):
                    emit_proj_group(xk[0], wk_sb, bk_sb, kT8, 0, i)
                for b in (1, 2, 3):
                    xk[b] = load_x_chunks(xkT, b)
                nc.sync.dma_start(wo_sb[:, 0, :], woT[0:128, :])
                nc.sync.dma_start(wo_sb[0:64, 1, :], woT[128:CL, :])

                # deferred ops drained into designated slots:
                # k-proj blocks 1..3 feed the first head's kt sweep just in
                # time; q-proj for sq 1..3 + their prefixes run mid-stream.
                bg_work = []
                for b in (1, 2, 3):
                    for i in range(2):
                        bg_work.append(
                            (6 * b - 4 + 2 * i,
                             lambda b=b, i=i: emit_proj_group(
                                 xk[b], wk_sb, bk_sb, kT8, b, i)))
                xq = {}
                for nb, sqb in ((40, 1), (136, 2), (232, 3)):
                    bg_work.append((nb, lambda s=sqb: xq.__setitem__(
                        s, load_x_chunks(xqT, s))))
                    for i in range(2):
                        bg_work.append(
                            (nb + 5 + 5 * i,
                             lambda s=sqb, i=i: emit_proj_group(
                                 xq[s], wq_sb, bq_sb, qT8, s, i)))
                    for h in range(HL):
                        bg_work.append(
                            (nb + 15 + 5 * h,
                             lambda s=sqb, h=h: emit_prefix(s, h)))

                slots = [(sq, h, kt)
                         for sq in range(NSQ)
                         for h in range(HL)
                         for kt in range(NKT)]
                pending = []

                def pop_one():
                    (s2, e2) = pending.pop(0)
                    emit_ctx(*s2, e2)

                vst = 0
                for j, slot in enumerate(slots):
                    expt = emit_scores_exp(*slot)
                    pending.append((slot, expt))
                    if vst < NST:
                        emit_vproj(vst)
                        vst += 1
                    trail_eff = TRAIL if j < len(slots) - 34 else 2
                    for _ in range(3):
                        if not pending:
                            break
                        need = (trail_eff + GAP if pending[0][0][2] == 0
                                else trail_eff)
                        if len(pending) > need:
                            pop_one()
                        else:
                            break
                    if bg_work and j >= bg_work[0][0]:
                        bg_work.pop(0)[1]()
                    elif outproj_work and j % 2 == 0:
                        # every other slot: out-proj rides the pss rotation
                        emit_outproj_tile(*outproj_work.pop(0))
                while pending:
                    pop_one()
                    if outproj_work:
                        emit_outproj_tile(*outproj_work.pop(0))
                for _, op in bg_work:
                    op()
                while outproj_work:
                    emit_outproj_tile(*outproj_work.pop(0))

    nc.compile()
    return nc


def _get_nc():
    with _lock:
        if "nc" not in _compiled:
            _compiled["nc"] = _build()
        return _compiled["nc"]


def _chan_perm():
    # fp8 DoubleRow layout: channel (p, i) <- head p//32, d = i*32 + p%32
    cols = np.empty((2, 96), np.int64)
    for i in range(2):
        for p in range(96):
            cols[i, p] = (p // 32) * 64 + i * 32 + (p % 32)
    return cols.reshape(-1)  # j = i*96 + p


def _prep_in_maps(query, key, value, prompt, Wq, bq, Wk, bk, Wv, bv, Wo, bo):
    f32 = np.float32
    qT = [np.ascontiguousarray(query[b].T).astype(BF16) for b in range(B)]
    kT = [np.ascontiguousarray(key[b].T).astype(BF16) for b in range(B)]
    vT = [np.ascontiguousarray(value[b].T).astype(BF16) for b in range(B)]
    perm = _chan_perm()
    in_maps = []
    for core in range(NCORES):
        b, g = core // NG, core % NG
        cs = slice(g * CL, (g + 1) * CL)
        Wq_g = np.asarray(Wq)[cs, :]
        Wk_g = np.asarray(Wk)[cs, :]
        bq_g = np.asarray(bq)[cs].astype(f32)
        bk_g = np.asarray(bk)[cs].astype(f32)
        kp = np.zeros((96, 2, PP), FP8)
        for i in range(2):
            for p in range(96):
                gh = g * HL + p // 32
                d = i * 32 + p % 32
                kp[p, i, :] = prompt[b, 0, :, gh, d].astype(FP8)
        vpa = np.zeros((PP, HL, D + 1), BF16)
        vpa[:, :, D] = 1.0
        for h in range(HL):
            gh = g * HL + h
            vpa[:, h, 0:D] = prompt[b, 1, :, gh, :].astype(BF16)
        in_maps.append({
            "xqT": qT[b], "xkT": kT[b], "xvT": vT[b],
            "wqT": np.ascontiguousarray(Wq_g[perm, :].T).astype(BF16),
            "wkT": np.ascontiguousarray(Wk_g[perm, :].T).astype(BF16),
            "wvT": np.ascontiguousarray(np.asarray(Wv)[cs, :].T).astype(BF16),
            "woT": np.ascontiguousarray(np.asarray(Wo)[:, cs].T).astype(BF16),
            "bq": np.ascontiguousarray(
                bq_g[perm].reshape(2, 96).T).astype(f32),
            "bk": np.ascontiguousarray(
                bk_g[perm].reshape(2, 96).T).astype(f32),
            "bv": np.ascontiguousarray(
                np.asarray(bv)[cs]).astype(f32).reshape(1, CL),
            "kp8": kp, "vp": vpa,
        })
    return in_maps


def _combine(results, bo):
    out = np.empty((B, S, E), np.float32)
    for b in range(B):
        acc = results[b * NG]["outT"].astype(np.float32)
        for g in range(1, NG):
            acc = acc + results[b * NG + g]["outT"]
        out[b] = acc.T
    if bo is not None and np.any(bo):
        out += np.asarray(bo, np.float32)
    return out


def run(inputs, trace=False):
    """Returns (output, exec_time_ns or None)."""
    from concourse import bass_utils

    nc = _get_nc()
    in_maps = _prep_in_maps(**{k: np.asarray(v) for k, v in inputs.items()})
    bo = np.asarray(inputs["bo"])
    res = bass_utils.run_bass_kernel_spmd(
        nc, in_maps, core_ids=list(range(NCORES)), trace=trace,
    )
    return _combine(res.results, bo), res.exec_time_ns


def kernel(**inputs):
    out, _ = run(inputs)
    return out


# revision 41
# speedup vs baseline: 1.0237x; 1.0237x over previous
"""Multi-head attention (B=2,S=4096,E=768,H=12,D=64 + 16-token K/V prompt
prefix) on 8 Trainium2 NeuronCores.

Sharding: 2 batches x 4 head-groups (3 heads each). Each core computes QKV
projections for its 3 heads, full attention over its batch, and a partial
output projection (its 192 ctx channels); the host sums the 4 partials per
batch.

v2 design (vs the 485us baseline, which was jointly PE- and ScalarE-bound):
  * scores matmuls run in fp8e4m3 with MatmulPerfMode.DoubleRow (d=64 split
    as [32 partitions x 2 interleave]); 0.5 cycles/row halves scores PE time.
    q/k live only in fp8; measured end-to-end rel-err impact ~1.2e-2.
  * ctx matmul is flipped: expt [k,128q] tiles are the *stationary* operand
    and v [k,65] the moving one, so each instruction streams 65 rows instead
    of 512 -- ctx PE time halves.  The ones-column in v still accumulates
    the softmax denominator (psc column 64).
  * exp is the 1/8-scaled softmax numerator; it is load-balanced across
    ScalarE (activation Exp, scale=1/8) AND Vector/Pool engines
    (tensor_tensor pow: expt = (e^{1/8})^s with a memset base tile).
  * ctx comes out of PSUM in [q, d] orientation; normalization is a single
    per-partition tensor_scalar divide; re-transposition to [d, q] for the
    out-projection rides the idle DMA engines via XBAR dma_start_transpose
    (two heads batched per transfer to satisfy the 128-col constraint).
  * PSUM: 2x[128,1024] scores + 1x[128,2,4,128pad] ctx + 2x[128,512]
    time-multiplexed (v-proj/bg q-proj/out-proj) = 8 banks exactly.
"""

import sys
import threading

import numpy as np

if "/opt/trn_rl_repo" not in sys.path:
    sys.path.insert(0, "/opt/trn_rl_repo")

import ml_dtypes

BF16 = ml_dtypes.bfloat16
FP8 = ml_dtypes.float8_e4m3

B, S, E, H, D, PP = 2, 4096, 768, 12, 64, 16
NCORES = 8
NG = 4          # head-groups (tensor parallel)
HL = H // NG    # 3 local heads
CL = HL * D     # 192 local channels
SKV = PP + S    # 4112
NKT = S // 128  # 32 full k-tiles (prefix handled separately)
QT = 1024       # q tile width for scores/exp/ctx
NSQ = S // QT   # 4
TRAIL = 12      # ctx matmuls trail scores by this many slots
NST = S // 128  # 32 v stiles
GAP = 6
# Schraudolph exp for the DVE share: bf16 bits of exp(s/8) ~=
# int16(s*SCHR_A + SCHR_B); one fused tensor_scalar (mult,add) writing
# through an int16 bitcast of the bf16 expt tile.  ~1.8% rms relative
# error on those tiles; the Act share stays exact, so total error scales
# with sqrt(phi).  C=7.5 centers the sawtooth; +0.5 makes trunc rounding.
SCHR_A = 128 * 1.4426950408889634 / 8   # 128*log2(e)/8
SCHR_B = 16256.5 - 7.5
# exp engine assignment pattern per slot: A=ScalarE (exact), D=Vector
# (Schraudolph).  GPSIMD cannot access PSUM; DVE has no transcendentals.
EXP_PAT = "ADADADADA"

_lock = threading.Lock()
_compiled = {}


def _build():
    import concourse.bass as bass  # noqa: F401
    import concourse.mybir as mybir
    import concourse.tile as tile
    from concourse import bacc

    f32 = mybir.dt.float32
    bf16 = mybir.dt.bfloat16
    fp8 = mybir.dt.float8e4
    i16 = mybir.dt.int16
    EXP = mybir.ActivationFunctionType.Exp
    IDN = mybir.ActivationFunctionType.Identity
    DIV = mybir.AluOpType.divide
    MUL = mybir.AluOpType.mult
    ADD = mybir.AluOpType.add
    DR = mybir.MatmulPerfMode.DoubleRow

    nc = bacc.Bacc("TRN2", target_bir_lowering=False, debug=False)

    xqT = nc.dram_tensor("xqT", [E, S], bf16, kind="ExternalInput").ap()
    xkT = nc.dram_tensor("xkT", [E, S], bf16, kind="ExternalInput").ap()
    xvT = nc.dram_tensor("xvT", [E, S], bf16, kind="ExternalInput").ap()
    wqT = nc.dram_tensor("wqT", [E, CL], bf16, kind="ExternalInput").ap()
    wkT = nc.dram_tensor("wkT", [E, CL], bf16, kind="ExternalInput").ap()
    wvT = nc.dram_tensor("wvT", [E, CL], bf16, kind="ExternalInput").ap()
    woT = nc.dram_tensor("woT", [CL, E], bf16, kind="ExternalInput").ap()
    bq = nc.dram_tensor("bq", [96, 2], f32, kind="ExternalInput").ap()
    bk = nc.dram_tensor("bk", [96, 2], f32, kind="ExternalInput").ap()
    bv = nc.dram_tensor("bv", [1, CL], f32, kind="ExternalInput").ap()
    kp8 = nc.dram_tensor("kp8", [96, 2, PP], fp8, kind="ExternalInput").ap()
    vp = nc.dram_tensor("vp", [PP, HL, D + 1], bf16, kind="ExternalInput").ap()
    outT = nc.dram_tensor("outT", [E, S], f32, kind="ExternalOutput").ap()

    with tile.TileContext(nc) as tc:
        with tc.tile_pool(name="persist", bufs=1) as pers:
            # q-projection weights/bias first: they gate the first matmuls
            wq_sb = pers.tile([128, 6, CL], bf16)
            nc.sync.dma_start(wq_sb[:], wqT.rearrange("(t p) c -> p t c", p=128))
            bq_sb = pers.tile([96, 2], f32)
            nc.sync.dma_start(bq_sb[:], bq[:])

            wk_sb = pers.tile([128, 6, CL], bf16)
            wv_sb = pers.tile([128, 6, CL], bf16)
            wo_sb = pers.tile([128, 2, E], bf16)
            bk_sb = pers.tile([96, 2], f32)
            bvb_sb = pers.tile([128, CL], f32)
            kp_sb = pers.tile([96, 2, PP], fp8)
            vp_sb = pers.tile([PP, HL, D + 1], bf16)

            # activations
            qT8 = pers.tile([96, 2, S], fp8)
            kT8 = pers.tile([96, 2, S], fp8)
            v_sb = pers.tile([128, NST, HL, D + 1], bf16)
            ctxT_sb = pers.tile([128, 2, S], bf16)
            expp_sb = pers.tile([PP, HL, S], bf16)  # prefix exp rows per head
            # normalized ctx staging, [q, d] orientation, manual sq-parity
            # double buffer; cn01 packs heads 0,1 so one XBAR dma transposes
            # 128 columns at once; cn2 pads head 2 with a junk half.
            cn01 = pers.tile([128, 2, 8, 2, D], bf16)
            cn2 = pers.tile([128, 2, 8, 2, D], bf16)

            nc.vector.memset(v_sb[:, :, :, D:D + 1], 1.0)
            nc.vector.memset(cn2[:], 0.0)

            # One unified stream phase.  PSUM: ps_s 3x[128,1024] (6 banks,
            # 3-deep rotation so the exp engines run back-to-back) + ps_c
            # 1x[128,2,4,128] (2 banks) = 8.  Projections / out-projection /
            # prefix borrow ps_s rotation slots (same tag+shape, partial use);
            # note matmul PSUM writes must stay within one 2KB bank, so every
            # matmul writes at most 512 f32 columns.
            with (
                tc.tile_pool(name="ps_s", bufs=3, space="PSUM") as ps_s,
                tc.tile_pool(name="ps_c", bufs=1, space="PSUM") as ps_c,
                tc.tile_pool(name="expt_pool", bufs=20) as expt_pool,
                tc.tile_pool(name="xv_pool", bufs=8) as xv_pool,
                tc.tile_pool(name="xq2_pool", bufs=26) as xq2_pool,
                tc.tile_pool(name="out_pool", bufs=3) as out_pool,
                tc.tile_pool(name="nrm_pool", bufs=2) as nrm_pool,
            ):
                expcnt = [0]

                def emit_exp(dst, src, exact=False):
                    eng = "A" if exact else EXP_PAT[expcnt[0] % len(EXP_PAT)]
                    expcnt[0] += 1
                    if eng == "A":
                        nc.scalar.activation(dst, src, EXP, scale=0.125)
                    else:
                        nc.vector.tensor_scalar(
                            dst.bitcast(i16), src, float(SCHR_A),
                            float(SCHR_B), MUL, ADD)

                def pss_tile(name):
                    return ps_s.tile([128, QT], f32, tag="pss", name=name)

                def load_x_chunks(xin, sq):
                    tiles = []
                    for ech in range(6):
                        xt = xq2_pool.tile([128, QT], bf16, tag="xt2",
                                           name="xt2")
                        nc.sync.dma_start(
                            xt[:],
                            xin[ech * 128:(ech + 1) * 128,
                                sq * QT:(sq + 1) * QT],
                        )
                        tiles.append(xt)
                    return tiles

                def emit_proj_group(xts, wsb, bsb, dst8, sq, i):
                    # one [96, 1024] projection result via two 512-col mms
                    p = pss_tile("pqk")
                    for n in range(2):
                        ns = slice(n * 512, (n + 1) * 512)
                        for ech in range(6):
                            nc.tensor.matmul(
                                p[0:96, ns],
                                wsb[:, ech, i * 96:(i + 1) * 96],
                                xts[ech][:, ns],
                                start=(ech == 0), stop=(ech == 5),
                            )
                    qs = slice(sq * QT, (sq + 1) * QT)
                    nc.scalar.activation(
                        dst8[0:96, i, qs], p[0:96, :], IDN,
                        bias=bsb[:, i:i + 1])

                def emit_prefix(sq, h):
                    hp = slice(32 * h, 32 * h + 32)
                    psp = pss_tile("psp")
                    for n in range(2):
                        ns = slice(n * 512, (n + 1) * 512)
                        qs = slice(sq * QT + n * 512, sq * QT + (n + 1) * 512)
                        nc.tensor.matmul(
                            psp[0:PP, ns], kp_sb[hp, :, :], qT8[hp, :, qs],
                            start=True, stop=True, perf_mode=DR,
                        )
                    emit_exp(expp_sb[:, h, sq * QT:(sq + 1) * QT],
                             psp[0:PP, :], exact=True)

                # xv DMA loads, one sq-group of 6 chunks at a time
                xvts = {}

                def load_xv(sqx):
                    tiles = []
                    for ech in range(6):
                        xvt = xv_pool.tile([128, QT], bf16, tag="xvt",
                                           name="xvt")
                        nc.sync.dma_start(
                            xvt[:],
                            xvT[ech * 128:(ech + 1) * 128,
                                sqx * QT:(sqx + 1) * QT],
                        )
                        tiles.append(xvt)
                    xvts[sqx] = tiles

                def emit_vproj(st):
                    sqx, stl = st // (QT // 128), st % (QT // 128)
                    # prefetch mid-group so the xv DMAs stay off the k-proj
                    # chunks' critical DMA window at stream start
                    if stl == 4 and sqx + 1 < NSQ:
                        load_xv(sqx + 1)
                    pv = pss_tile("pv")
                    for ech in range(6):
                        nc.tensor.matmul(
                            pv[:, 0:CL],
                            xvts[sqx][ech][:, stl * 128:(stl + 1) * 128],
                            wv_sb[:, ech, :],
                            start=(ech == 0), stop=(ech == 5),
                        )
                    nc.vector.tensor_add(
                        v_sb[:, st, :, 0:D],
                        pv[:, 0:CL].rearrange("p (h d) -> p h d", h=HL),
                        bvb_sb[:].rearrange("p (h d) -> p h d", h=HL),
                    )
                    if stl == (QT // 128) - 1:
                        del xvts[sqx]

                def emit_scores_exp(sq, h, kt):
                    hp = slice(32 * h, 32 * h + 32)
                    expt = expt_pool.tile([128, QT], bf16, tag="expt",
                                          name="expt")
                    pss = pss_tile("pss")
                    for n in range(2):
                        ns = slice(n * 512, (n + 1) * 512)
                        qs = slice(sq * QT + n * 512, sq * QT + (n + 1) * 512)
                        nc.tensor.matmul(
                            pss[:, ns],
                            kT8[hp, :, kt * 128:(kt + 1) * 128],
                            qT8[hp, :, qs],
                            start=True, stop=True, perf_mode=DR,
                        )
                    emit_exp(expt[:], pss[:])
                    return expt

                psc_tiles = {}
                outproj_work = []

                def emit_ctx(sq, h, kt, expt):
                    key = (sq, h)
                    if kt == 0:
                        psc_tiles[key] = ps_c.tile(
                            [128, 2, 4, 128], f32, tag="psc", name="psc")
                    psc = psc_tiles[key]
                    # PSUM zero-region (2KB bank) semantics: only the first
                    # slice per bank may carry start=True (it marks the whole
                    # region pending-zero; sibling slices' first writes then
                    # overwrite-on-first-touch), and only the last slice may
                    # carry stop=True (it clears the whole region's group).
                    for qb in range(8):
                        nc.tensor.matmul(
                            psc[:, qb // 4, qb % 4, 0:D + 1],
                            expt[:, qb * 128:(qb + 1) * 128],
                            v_sb[:, kt, h, :],
                            start=(kt == 0 and qb % 4 == 0),
                            stop=(kt == NKT - 1 and qb % 4 == 3),
                        )
                    if kt == TRAIL - 1:
                        # prompt-prefix ctx contribution (reads expp_sb rows)
                        for qb in range(8):
                            qs = slice(sq * QT + qb * 128,
                                       sq * QT + (qb + 1) * 128)
                            nc.tensor.matmul(
                                psc[:, qb // 4, qb % 4, 0:D + 1],
                                expp_sb[:, h, qs],
                                vp_sb[:, h, :],
                                start=False, stop=False,
                            )
                    if kt == NKT - 1:
                        emit_norm(sq, h, psc)
                        del psc_tiles[key]

                def emit_norm(sq, h, psc):
                    par = sq % 2
                    cn = cn2 if h == 2 else cn01
                    hh = 0 if h == 2 else h
                    # hw tensor_scalar has no divide: batched reciprocal of
                    # the 8 denominator columns, then per-block multiplies,
                    # alternating Act/DVE so neither engine queue bursts
                    rc = nrm_pool.tile([128, 8], f32, tag="rc", name="rc")
                    nc.vector.reciprocal(
                        rc[:].rearrange("p (a b) -> p a b", a=2),
                        psc[:, :, :, D:D + 1].squeeze(3))
                    for qb in range(8):
                        if qb % 2 == 0:
                            nc.scalar.activation(
                                cn[:, par, qb, hh, :],
                                psc[:, qb // 4, qb % 4, 0:D],
                                IDN, scale=rc[:, qb:qb + 1])
                        else:
                            nc.vector.tensor_scalar(
                                cn[:, par, qb, hh, :],
                                psc[:, qb // 4, qb % 4, 0:D],
                                rc[:, qb:qb + 1],
                                None, MUL,
                            )
                    if h >= 1:
                        # heads 0,1 pair (after h1) / head 2 -> XBAR transpose
                        cnin, pr = (cn01, 0) if h == 1 else (cn2, 1)
                        for qb in range(8):
                            qs = slice(sq * QT + qb * 128,
                                       sq * QT + (qb + 1) * 128)
                            nc.sync.dma_start_transpose(
                                ctxT_sb[:, pr, qs], cnin[:, par, qb, :, :])
                    if h == HL - 1:
                        for et in range(6):
                            for n in range(2):
                                outproj_work.append((et, sq, n))

                def emit_outproj_tile(et, sq, n):
                    es = slice(et * 128, (et + 1) * 128)
                    qs = slice(sq * QT + n * 512, sq * QT + (n + 1) * 512)
                    po3 = pss_tile("po3")
                    nc.tensor.matmul(
                        po3[:, 0:512], wo_sb[:, 0, es], ctxT_sb[:, 0, qs],
                        start=True, stop=False,
                    )
                    nc.tensor.matmul(
                        po3[:, 0:512], wo_sb[0:64, 1, es],
                        ctxT_sb[0:64, 1, qs],
                        start=False, stop=True,
                    )
                    # DMA cannot read PSUM: stage through SBUF via DVE
                    ot = out_pool.tile([128, 512], f32, tag="ot", name="ot")
                    nc.vector.tensor_copy(ot[:], po3[:, 0:512])
                    nc.sync.dma_start(outT[es, qs], ot[:])

                # ---- stream startup: q-proj(sq0), prefix(sq0), k-proj(0) ----
                # DMA issue order matters: the (serial) DMA engines gate the
                # startup ramp.  xq0/xk0 first (they gate the first exps),
                # then wv/vp/xv-side, then the remaining xk blocks; wo last.
                xq0 = load_x_chunks(xqT, 0)
                nc.sync.dma_start(
                    wk_sb[:], wkT.rearrange("(t p) c -> p t c", p=128))
                nc.sync.dma_start(bk_sb[:], bk[:])
                nc.sync.dma_start(kp_sb[:], kp8[:])
                for i in range(2):
                    emit_proj_group(xq0, wq_sb, bq_sb, qT8, 0, i)
                xk = {0: load_x_chunks(xkT, 0)}
                nc.sync.dma_start(
                    wv_sb[:], wvT.rearrange("(t p) c -> p t c", p=128))
                nc.sync.dma_start(bvb_sb[:], bv.to_broadcast((128, CL)))
                nc.sync.dma_start(vp_sb[:], vp[:])
                for h in range(HL):
                    emit_prefix(0, h)
                for i in range(2):
                    emit_proj_group(xk[0], wk_sb, bk_sb, kT8, 0, i)
                load_xv(0)
                for b in (1, 2, 3):
                    xk[b] = load_x_chunks(xkT, b)
                nc.sync.dma_start(wo_sb[:, 0, :], woT[0:128, :])
                nc.sync.dma_start(wo_sb[0:64, 1, :], woT[128:CL, :])

                # deferred ops drained into designated slots:
                # k-proj blocks 1..3 feed the first head's kt sweep just in
                # time; q-proj for sq 1..3 + their prefixes run mid-stream.
                bg_work = []
                for b in (1, 2, 3):
                    for i in range(2):
                        bg_work.append(
                            (6 * b - 4 + 2 * i,
                             lambda b=b, i=i: emit_proj_group(
                                 xk[b], wk_sb, bk_sb, kT8, b, i)))
                xq = {}
                for nb, sqb in ((40, 1), (136, 2), (232, 3)):
                    bg_work.append((nb, lambda s=sqb: xq.__setitem__(
                        s, load_x_chunks(xqT, s))))
                    for i in range(2):
                        bg_work.append(
                            (nb + 5 + 5 * i,
                             lambda s=sqb, i=i: emit_proj_group(
                                 xq[s], wq_sb, bq_sb, qT8, s, i)))
                    for h in range(HL):
                        bg_work.append(
                            (nb + 15 + 5 * h,
                             lambda s=sqb, h=h: emit_prefix(s, h)))

                slots = [(sq, h, kt)
                         for sq in range(NSQ)
                         for h in range(HL)
                         for kt in range(NKT)]
                pending = []

                def pop_one():
                    (s2, e2) = pending.pop(0)
                    emit_ctx(*s2, e2)

                vst = 0
                for j, slot in enumerate(slots):
                    expt = emit_scores_exp(*slot)
                    pending.append((slot, expt))
                    # vproj starts at slot 8: the in-order PE queue must not
                    # hit vproj matmuls before their xv chunks can have landed
                    if vst < NST and j >= 8:
                        emit_vproj(vst)
                        vst += 1
                    trail_eff = TRAIL if j < len(slots) - 34 else 2
                    for _ in range(3):
                        if not pending:
                            break
                        need = (trail_eff + GAP if pending[0][0][2] == 0
                                else trail_eff)
                        if len(pending) > need:
                            pop_one()
                        else:
                            break
                    if bg_work and j >= bg_work[0][0]:
                        bg_work.pop(0)[1]()
                    elif outproj_work and j % 2 == 0:
                        # every other slot: out-proj rides the pss rotation
                        emit_outproj_tile(*outproj_work.pop(0))
                while pending:
                    pop_one()
                    if outproj_work:
                        emit_outproj_tile(*outproj_work.pop(0))
                for _, op in bg_work:
                    op()
                while outproj_work:
                    emit_outproj_tile(*outproj_work.pop(0))

    nc.compile()
    return nc


def _get_nc():
    with _lock:
        if "nc" not in _compiled:
            _compiled["nc"] = _build()
        return _compiled["nc"]


def _chan_perm():
    # fp8 DoubleRow layout: channel (p, i) <- head p//32, d = i*32 + p%32
    cols = np.empty((2, 96), np.int64)
    for i in range(2):
        for p in range(96):
            cols[i, p] = (p // 32) * 64 + i * 32 + (p % 32)
    return cols.reshape(-1)  # j = i*96 + p


def _prep_in_maps(query, key, value, prompt, Wq, bq, Wk, bk, Wv, bv, Wo, bo):
    f32 = np.float32
    qT = [np.ascontiguousarray(query[b].T).astype(BF16) for b in range(B)]
    kT = [np.ascontiguousarray(key[b].T).astype(BF16) for b in range(B)]
    vT = [np.ascontiguousarray(value[b].T).astype(BF16) for b in range(B)]
    perm = _chan_perm()
    in_maps = []
    for core in range(NCORES):
        b, g = core // NG, core % NG
        cs = slice(g * CL, (g + 1) * CL)
        Wq_g = np.asarray(Wq)[cs, :]
        Wk_g = np.asarray(Wk)[cs, :]
        bq_g = np.asarray(bq)[cs].astype(f32)
        bk_g = np.asarray(bk)[cs].astype(f32)
        kp = np.zeros((96, 2, PP), FP8)
        for i in range(2):
            for p in range(96):
                gh = g * HL + p // 32
                d = i * 32 + p % 32
                kp[p, i, :] = prompt[b, 0, :, gh, d].astype(FP8)
        vpa = np.zeros((PP, HL, D + 1), BF16)
        vpa[:, :, D] = 1.0
        for h in range(HL):
            gh = g * HL + h
            vpa[:, h, 0:D] = prompt[b, 1, :, gh, :].astype(BF16)
        in_maps.append({
            "xqT": qT[b], "xkT": kT[b], "xvT": vT[b],
            "wqT": np.ascontiguousarray(Wq_g[perm, :].T).astype(BF16),
            "wkT": np.ascontiguousarray(Wk_g[perm, :].T).astype(BF16),
            "wvT": np.ascontiguousarray(np.asarray(Wv)[cs, :].T).astype(BF16),
            "woT": np.ascontiguousarray(np.asarray(Wo)[:, cs].T).astype(BF16),
            "bq": np.ascontiguousarray(
                bq_g[perm].reshape(2, 96).T).astype(f32),
            "bk": np.ascontiguousarray(
                bk_g[perm].reshape(2, 96).T).astype(f32),
            "bv": np.ascontiguousarray(
                np.asarray(bv)[cs]).astype(f32).reshape(1, CL),
            "kp8": kp, "vp": vpa,
        })
    return in_maps


def _combine(results, bo):
    out = np.empty((B, S, E), np.float32)
    for b in range(B):
        acc = results[b * NG]["outT"].astype(np.float32)
        for g in range(1, NG):
            acc = acc + results[b * NG + g]["outT"]
        out[b] = acc.T
    if bo is not None and np.any(bo):
        out += np.asarray(bo, np.float32)
    return out


def run(inputs, trace=False):
    """Returns (output, exec_time_ns or None)."""
    from concourse import bass_utils

    nc = _get_nc()
    in_maps = _prep_in_maps(**{k: np.asarray(v) for k, v in inputs.items()})
    bo = np.asarray(inputs["bo"])
    res = bass_utils.run_bass_kernel_spmd(
        nc, in_maps, core_ids=list(range(NCORES)), trace=trace,
    )
    return _combine(res.results, bo), res.exec_time_ns


def kernel(**inputs):
    out, _ = run(inputs)
    return out
